# revision 3
# baseline (speedup 1.0000x reference)
# Trainium2 raw-Bass kernel for nn_GraphStack (gnn_message_passing).
#
# Math per layer (B=8, N=2048, F=128, L=2):
#   w1 = lrelu(x @ W3); w2 = lrelu(x @ W4)
#   S = w2^T x ; diag_i = w1_i . w2_i
#   msg = (w1 @ S - diag*x)/(N-1) ; x = lrelu(msg @ W5) + x
#
# Everything is kept in two SBUF layouts, both filled/drained with purely
# CONTIGUOUS DMA (strided DRAM patterns cost ~1.8 ms per descriptor set on
# this stack):
#   xT [128f, 2048n]  - transposed, host-prepared
#   xA [128p, 16c*128f] - x row-major loaded as-is; node n = 16p+c, so the
#       slice [:, c, :] is a valid 128-node natural chunk for contractions
#       over nodes (any node partition works for sums over all nodes).
# Output is produced transposed (yT) and transposed back on host.
#
# Diag correction: prod = -(w1T*w2T) elementwise; column sums via a
# ones-vector matmul (reduce), then broadcast back over partitions via a
# ones-row matmul; both have constant stationary operands. This replaces the
# 16 PE tile-transposes of the z-term formulation.
#
# 1/(N-1) is folded into W5 host-side. Biases are zeros by spec fill.
#
# Sharding: data-parallel, batch element b -> core b, no collectives.
#
# Raw Bass (not Tile): this container's walrus rejects instructions with
# more than one attached sync-wait; every cross-engine dependency is a
# standalone wait_ge on one of four monotonic semaphores.

import numpy as np
from contextlib import ExitStack

import concourse.bass as bass
import concourse.mybir as mybir
from concourse.bass_utils import run_bass_kernel_spmd

B, N, F, L = 8, 2048, 128, 2
NCH = N // 128
SLOPE = 0.1
FP = mybir.dt.float32
AF = mybir.ActivationFunctionType
ALU = mybir.AluOpType
ts = bass.ts

_CACHE = {}

# wconst layout: [W3_0 W4_0 W5'_0 W3_1 W4_1 W5'_1 ones] each [128,128]
WC_COLS = (3 * L + 1) * F


def emit(R=1):
    """Build the Bass module with R serialized copies of the per-call
    pipeline (R>1 is used by bench.py for loop-slope timing)."""
    nc = bass.Bass()

    xa_d = nc.declare_dram_parameter("xa", [N, F], FP, isOutput=False)
    xt_d = nc.declare_dram_parameter("xt", [F, N], FP, isOutput=False)
    wc_d = nc.declare_dram_parameter("wc", [F, WC_COLS], FP, isOutput=False)
    yt_d = nc.declare_dram_parameter("yt", [F, N], FP, isOutput=True)

    ctx = ExitStack()
    sb = lambda shape, name: ctx.enter_context(nc.sbuf_tensor(name, shape, FP))
    wc = sb([F, WC_COLS], "wc_sb")
    # xTA double buffer: [:, 0:N] = xA (natural interleaved), [:, N:2N] = xT
    xTA = [sb([128, 2 * N], f"xTA{i}") for i in range(2)]
    w12T = sb([128, 2 * N], "w12T")     # [:, 0:N] = w1T, [:, N:2N] = w2T
    w2A = sb([128, N], "w2A")
    prodneg = sb([128, N], "prodneg")
    ndT = sb([1, N], "ndT")             # -diag, transposed row
    zT = sb([128, N], "zT")
    msgT = sb([128, N], "msgT")
    S_sb = sb([F, F], "S_sb")
    ps = ctx.enter_context(nc.psum_tensor("ps", [128, 2 * N], FP))

    s_dma = ctx.enter_context(nc.semaphore("s_dma"))
    s_pe = ctx.enter_context(nc.semaphore("s_pe"))
    s_act = ctx.enter_context(nc.semaphore("s_act"))
    s_dve = ctx.enter_context(nc.semaphore("s_dve"))

    W3 = [wc[:, (3 * l + 0) * F : (3 * l + 1) * F] for l in range(L)]
    W4 = [wc[:, (3 * l + 1) * F : (3 * l + 2) * F] for l in range(L)]
    W5 = [wc[:, (3 * l + 2) * F : (3 * l + 3) * F] for l in range(L)]
    ones = wc[:, 3 * L * F : 3 * L * F + F]

    DMA_PER_ITER = 4 * 16  # wc, xa, xt loads + yt store
    LOADS = 3 * 16

    # ---- milestone numbering (python-side counters) ----
    pe_c, act_c, dve_c = [0], [0], [0]

    def nxt(c):
        c[0] += 1
        return c[0]

    M = {}
    for r in range(R):
        for l in range(L):
            last = l == L - 1
            for k in ("a", "c", "h", "e", "j", "l", "n"):
                M[f"pe_{k}{l}@{r}"] = nxt(pe_c)
            if not last:
                M[f"pe_p{l}@{r}"] = nxt(pe_c)
            for k in ("b", "d", "i", "f", "EF"):
                M[f"a_{k}{l}@{r}"] = nxt(act_c)
            for k in ("g", "m", "res"):
                M[f"d_{k}{l}@{r}"] = nxt(dve_c)

    def chunk_view(t2d):
        # [128, N] -> [128, c, p] strided view selecting nodes 16p+c
        return t2d.rearrange("g (p c) -> g c p", c=NCH)

    with nc.Block() as block:

        @block.sync
        def _(sync):
            for r in range(R):
                if r > 0:
                    sync.wait_ge(s_dma, r * DMA_PER_ITER)
                sync.dma_start(out=wc[:], in_=wc_d[:]).then_inc(s_dma, 16)
                sync.dma_start(out=xTA[0][:, 0:N], in_=xa_d.rearrange("(p a) f -> p (a f)", p=128)
                               ).then_inc(s_dma, 16)
                sync.dma_start(out=xTA[0][:, N : 2 * N], in_=xt_d[:]).then_inc(s_dma, 16)
                sync.wait_ge(s_dve, M[f"d_res{L-1}@{r}"])
                sync.dma_start(out=yt_d[:], in_=xTA[L % 2][:, N : 2 * N]).then_inc(s_dma, 16)
            sync.wait_ge(s_dma, R * DMA_PER_ITER)

        @block.tensor
        def _(tensor):
            for r in range(R):
                for l in range(L):
                    last = l == L - 1
                    cur, nxt_ = xTA[l % 2], xTA[(l + 1) % 2]
                    xA = cur[:, 0:N]
                    xT = cur[:, N : 2 * N]

                    # a: w12T_ps = [W3|W4]^T xT  (const stationary)
                    if l == 0:
                        tensor.wait_ge(s_dma, r * DMA_PER_ITER + LOADS)
                        if r > 0:
                            tensor.wait_ge(s_dve, M[f"d_res{L-1}@{r-1}"])
                    else:
                        tensor.wait_ge(s_dve, M[f"d_res{l-1}@{r}"])
                    for k in range(4):
                        nc.tensor.matmul(ps[:, ts(k, 512)], W3[l], xT[:, ts(k, 512)],
                                         start=True, stop=True)
                    for k in range(4):
                        mm = nc.tensor.matmul(ps[:, N + k * 512 : N + (k + 1) * 512], W4[l], xT[:, ts(k, 512)],
                                              start=True, stop=True)
                    mm.then_inc(s_pe, 1)  # pe_a

                    # c: w2A_ps = per-chunk x @ W4 (stationary = xT chunk views)
                    tensor.wait_ge(s_act, M[f"a_b{l}@{r}"])
                    xTc = chunk_view(xT)
                    for c in range(NCH):
                        mm = nc.tensor.matmul(ps[:, ts(c, 128)], xTc[:, c, :], W4[l],
                                              start=True, stop=True)
                    mm.then_inc(s_pe, 1)  # pe_c

                    # h: ndT_ps[1, N] = ones_col^T @ prodneg (column sums)
                    tensor.wait_ge(s_dve, M[f"d_g{l}@{r}"])
                    for k in range(4):
                        mm = nc.tensor.matmul(ps[0:1, N + k * 512 : N + (k + 1) * 512], ones[:, 0:1],
                                              prodneg[:, ts(k, 512)], start=True, stop=True)
                    mm.then_inc(s_pe, 1)  # pe_h

                    # e: S = sum_c w2A_c^T @ xA_c  (accumulating)
                    tensor.wait_ge(s_act, M[f"a_d{l}@{r}"])
                    w2Ac = w2A[:].rearrange("p (c f) -> p c f", f=F)
                    xAc = xA.rearrange("p (c f) -> p c f", f=F)
                    for c in range(NCH):
                        mm = nc.tensor.matmul(ps[:, 0:128], w2Ac[:, c, :], xAc[:, c, :],
                                              start=(c == 0), stop=(c == NCH - 1))
                    mm.then_inc(s_pe, 1)  # pe_e

                    # j: broadcast -diag across partitions: ones_row^T @ ndT
                    tensor.wait_ge(s_act, M[f"a_i{l}@{r}"])
                    for k in range(4):
                        mm = nc.tensor.matmul(ps[:, N + k * 512 : N + (k + 1) * 512], ones[0:1, :],
                                              ndT[0:1, ts(k, 512)], start=True, stop=True)
                    mm.then_inc(s_pe, 1)  # pe_j

                    # l: msgT_ps = S^T-contraction: lhsT=S_sb, moving w1T
                    tensor.wait_ge(s_act, M[f"a_f{l}@{r}"])
                    for k in range(4):
                        mm = nc.tensor.matmul(ps[:, ts(k, 512)], S_sb[:],
                                              w12T[:, ts(k, 512)], start=True, stop=True)
                    mm.then_inc(s_pe, 1)  # pe_l

                    # n: yT_ps = W5'^T msgT  (const stationary)
                    tensor.wait_ge(s_dve, M[f"d_m{l}@{r}"])
                    for k in range(4):
                        mm = nc.tensor.matmul(ps[:, N + k * 512 : N + (k + 1) * 512], W5[l],
                                              msgT[:, ts(k, 512)], start=True, stop=True)
                    mm.then_inc(s_pe, 1)  # pe_n

                    # p: yA_ps = per-chunk msg @ W5' (inner layers only)
                    if not last:
                        mTc = chunk_view(msgT[:])
                        for c in range(NCH):
                            mm = nc.tensor.matmul(ps[:, ts(c, 128)], mTc[:, c, :], W5[l],
                                                  start=True, stop=True)
                        mm.then_inc(s_pe, 1)  # pe_p

        @block.scalar
        def _(scalar):
            for r in range(R):
                for l in range(L):
                    last = l == L - 1
                    # b: w12T = Prelu(ps[0:2N])
                    scalar.wait_ge(s_pe, M[f"pe_a{l}@{r}"])
                    nc.scalar.activation(w12T[:], ps[:], AF.Prelu, alpha=SLOPE
                                         ).then_inc(s_act, 1)
                    # d: w2A = Prelu(ps[0:N])
                    scalar.wait_ge(s_pe, M[f"pe_c{l}@{r}"])
                    nc.scalar.activation(w2A[:], ps[:, 0:N], AF.Prelu, alpha=SLOPE
                                         ).then_inc(s_act, 1)
                    # i: ndT = copy(ps[0:1, N:2N])
                    scalar.wait_ge(s_pe, M[f"pe_h{l}@{r}"])
                    nc.scalar.activation(ndT[:], ps[0:1, N : 2 * N], AF.Copy
                                         ).then_inc(s_act, 1)
                    # f: S_sb = copy(ps[:, 0:128])
                    scalar.wait_ge(s_pe, M[f"pe_e{l}@{r}"])
                    nc.scalar.activation(S_sb[:], ps[:, 0:128], AF.Copy
                                         ).then_inc(s_act, 1)
                    # EF: Prelu in place over yA|yT (inner) or yT only (last)
                    if last:
                        scalar.wait_ge(s_pe, M[f"pe_n{l}@{r}"])
                        nc.scalar.activation(ps[:, N : 2 * N], ps[:, N : 2 * N],
                                             AF.Prelu, alpha=SLOPE).then_inc(s_act, 1)
                    else:
                        scalar.wait_ge(s_pe, M[f"pe_p{l}@{r}"])
                        nc.scalar.activation(ps[:], ps[:], AF.Prelu, alpha=SLOPE
                                             ).then_inc(s_act, 1)

        @block.vector
        def _(vector):
            for r in range(R):
                for l in range(L):
                    last = l == L - 1
                    cur, nxt_ = xTA[l % 2], xTA[(l + 1) % 2]
                    xT = cur[:, N : 2 * N]
                    # g: prodneg = -(w1T * w2T)
                    vector.wait_ge(s_act, M[f"a_b{l}@{r}"])
                    op = nc.vector.scalar_tensor_tensor(prodneg[:], w12T[:, 0:N], -1.0,
                                                        w12T[:, N : 2 * N],
                                                        op0=ALU.mult, op1=ALU.mult)
                    nc.vector.drain()
                    op.then_inc(s_dve, 1)  # d_g
                    # k: zT = ndB_ps * xT
                    vector.wait_ge(s_pe, M[f"pe_j{l}@{r}"])
                    nc.vector.tensor_mul(zT[:], ps[:, N : 2 * N], xT)
                    nc.vector.drain()
                    # m: msgT = msgT_ps + zT
                    vector.wait_ge(s_pe, M[f"pe_l{l}@{r}"])
                    op = nc.vector.scalar_tensor_tensor(msgT[:], ps[:, 0:N], 1.0, zT[:],
                                                        op0=ALU.mult, op1=ALU.add)
                    nc.vector.drain()
                    op.then_inc(s_dve, 1)  # d_m
                    # res: next = Prelu(psEF) + cur  (both halves inner; xT half last)
                    vector.wait_ge(s_act, M[f"a_EF{l}@{r}"])
                    if last:
                        op = nc.vector.scalar_tensor_tensor(nxt_[:, N : 2 * N],
                                                            ps[:, N : 2 * N], 1.0, xT,
                                                            op0=ALU.mult, op1=ALU.add)
                    else:
                        op = nc.vector.scalar_tensor_tensor(nxt_[:], ps[:], 1.0, cur[:],
                                                            op0=ALU.mult, op1=ALU.add)
                    nc.vector.drain()
                    op.then_inc(s_dve, 1)  # d_res

    ctx.close()
    return nc


def _pack_wconst(W3, W4, W5):
    w5s = W5 / (N - 1)
    blocks = []
    for l in range(L):
        blocks += [W3[l], W4[l], w5s[l]]
    blocks.append(np.ones((F, F), dtype=np.float32))
    return np.ascontiguousarray(np.concatenate(blocks, axis=1).astype(np.float32))


def kernel(x, W3, b3, W4, b4, W5, b5, _trace=False):
    x = np.asarray(x, dtype=np.float32)
    W3 = np.asarray(W3, dtype=np.float32)
    W4 = np.asarray(W4, dtype=np.float32)
    W5 = np.asarray(W5, dtype=np.float32)

    if "nc" not in _CACHE:
        _CACHE["nc"] = emit(1)
    nc = _CACHE["nc"]

    wcv = _pack_wconst(W3, W4, W5)
    in_maps = []
    for b in range(B):
        in_maps.append(
            {
                "xa": np.ascontiguousarray(x[b]),
                "xt": np.ascontiguousarray(x[b].T),
                "wc": wcv,
            }
        )
    res = run_bass_kernel_spmd(nc, in_maps, list(range(B)), trace=_trace)
    out = np.stack([res.results[b]["yt"].T for b in range(B)], axis=0)
    if _trace:
        return out, res
    return out
